# revision 1
# baseline (speedup 1.0000x reference)
"""Trainium2 Bass kernel for a 2-layer GRU (B=64, T=2048, I=16, H=256) + MLP regressor.

Strategy:
  - Data parallel: batch 64 sharded as 8 sequences per NeuronCore.
  - Each core runs BOTH GRU layers, software-pipelined with a D-step skew so the
    two layers' serial gate chains hide under each other's TensorE work.
  - Layout: gates-on-partitions. Recurrent matmul per step/layer:
    psum[g % 128 -> partitions, batch -> free] = Whh.T tiles (stationary) x h.T
    (moving, N=8). 6 M-chunks x 2 K-chunks = 12 matmuls/step/layer.
  - Input-gate projections (x@Wih.T / h1@Wih1.T) are precomputed in C-step
    chunks as wide matmuls (amortized), stored in step-major SBUF rings.
  - Regressor (Linear+ReLU+Linear+ReLU) runs fused every C steps on h2 history.
"""

import os
import sys

import numpy as np

if "/opt/trn_rl_repo" not in sys.path:
    sys.path.insert(0, "/opt/trn_rl_repo")

import concourse.bacc as bacc
import concourse.mybir as mybir
import concourse.tile as tile
from concourse.bass import ds, ts
from concourse.bass_utils import run_bass_kernel_spmd

# Problem constants (hardcoded per harness contract)
B_TOTAL = 64
N_CORES = 8
Bc = B_TOTAL // N_CORES  # 8 sequences per core
T = 2048
I_DIM = 16
H = 256
G = 3 * H  # 768 gate rows
C = 64  # chunk size for batched precomputes
S = 128  # ring size in steps (2 chunks)
D = 128  # layer-1 skew (steps)

F32 = mybir.dt.float32
BF16 = mybir.dt.bfloat16

AF = mybir.ActivationFunctionType


def build_program(dt_compute=F32, repeat=1):
    """Build + compile the SPMD program (identical on all 8 cores)."""
    DT = dt_compute
    nc = bacc.Bacc("TRN2", target_bir_lowering=False, debug=False,
                   num_devices=N_CORES)

    # ---- DRAM I/O ----
    # aux (amortized) matmuls are always fp32; only the per-step recurrent
    # matmul uses DT.
    xT_h = nc.dram_tensor("xT", [I_DIM + 1, T * Bc], F32, kind="ExternalInput")
    wh0_h = nc.dram_tensor("wh0T", [H, G], DT, kind="ExternalInput")
    wih0_h = nc.dram_tensor("wih0T", [I_DIM + 1, G], F32, kind="ExternalInput")
    wh1_h = nc.dram_tensor("wh1T", [H, G], DT, kind="ExternalInput")
    wih1_h = nc.dram_tensor("wih1T", [H, G], F32, kind="ExternalInput")
    w1_h = nc.dram_tensor("w1T", [H, H], F32, kind="ExternalInput")
    b1_h = nc.dram_tensor("b1c", [128, 2], F32, kind="ExternalInput")
    w2_h = nc.dram_tensor("w2c", [128, 2], F32, kind="ExternalInput")
    b2_h = nc.dram_tensor("b2c", [1, 1], F32, kind="ExternalInput")
    out_h = nc.dram_tensor("out", [T // C, C * Bc], F32, kind="ExternalOutput")

    NB = Bc  # batch per core
    W = NB * 2  # 16: one h-state slot width (2 k-chunks x 8)
    GW = 6 * NB  # 48: gate block width per step

    with tile.TileContext(nc) as tc:
        with (
            tc.tile_pool(name="cst", bufs=1) as cst,
            tc.tile_pool(name="work", bufs=3) as work,
            tc.tile_pool(name="pg", bufs=2, space="PSUM") as pg,
            tc.tile_pool(name="pbig", bufs=3, space="PSUM") as pbig,
            tc.tile_pool(name="pst2", bufs=1, space="PSUM") as pst2,
        ):
            # ---- persistent SBUF ----
            xT = cst.tile([I_DIM + 1, T * NB], F32, tag="xT")
            wh0 = cst.tile([128, 12 * 128], DT, tag="wh0")
            wh1 = cst.tile([128, 12 * 128], DT, tag="wh1")
            wih0 = cst.tile([I_DIM + 1, G], F32, tag="wih0")
            wih1 = cst.tile([128, 2 * G], F32, tag="wih1")
            w1 = cst.tile([128, 4 * 128], F32, tag="w1")
            w2 = cst.tile([128, 2], F32, tag="w2")
            b1 = cst.tile([128, 2], F32, tag="b1")
            b2 = cst.tile([1, 1], F32, tag="b2")
            xg0 = cst.tile([128, S * GW], F32, tag="xg0")  # step-major ring
            xg1 = cst.tile([128, S * GW], F32, tag="xg1")
            h1h = cst.tile([128, S * W], DT, tag="h1h")  # h1.T history ring
            h2h = cst.tile([128, S * W], DT, tag="h2h")
            # fp32 master copy of the hidden state (bf16 mode only): the
            # recurrent accumulation stays fp32; bf16 is only a matmul cast.
            state_f32 = DT != F32
            if state_f32:
                h1f = cst.tile([128, S * W], F32, tag="h1f")
                h2f = cst.tile([128, S * W], F32, tag="h2f")
            else:
                h1f, h2f = h1h, h2h

            # ---- load constants ----
            nc.sync.dma_start(xT[:], xT_h[:, :])
            for m in range(6):
                for k in range(2):
                    i = m * 2 + k
                    nc.sync.dma_start(wh0[:, ts(i, 128)],
                                      wh0_h[ds(k * 128, 128), ds(m * 128, 128)])
                    nc.sync.dma_start(wh1[:, ts(i, 128)],
                                      wh1_h[ds(k * 128, 128), ds(m * 128, 128)])
            nc.sync.dma_start(wih0[:], wih0_h[:, :])
            for k in range(2):
                nc.sync.dma_start(wih1[:, ts(k, G)], wih1_h[ds(k * 128, 128), :])
            for mm in range(2):
                for k in range(2):
                    nc.sync.dma_start(w1[:, ts(mm * 2 + k, 128)],
                                      w1_h[ds(k * 128, 128), ds(mm * 128, 128)])
            nc.sync.dma_start(w2[:], w2_h[:, :])
            nc.sync.dma_start(b1[:], b1_h[:, :])
            nc.sync.dma_start(b2[:], b2_h[:, :])

            def emit_xg0_chunk(c):
                """xg0 for steps [c*C, (c+1)*C): one K<=17 matmul per M-chunk."""
                base_step = (c * C) % S
                for m in range(6):
                    ps = pbig.tile([128, C * NB], F32, tag="big")
                    nc.tensor.matmul(ps[:],
                                     wih0[:, ts(m, 128)],
                                     xT[:, ds(c * C * NB, C * NB)],
                                     start=True, stop=True)
                    # strided store into step-major ring: [128, (C steps, NB)]
                    dst = xg0[:, ds(base_step * GW, C * GW)]
                    dst = dst.rearrange("p (s g) -> p s g", g=GW)
                    dst = dst[:, :, ds(m * NB, NB)]
                    src = ps[:].rearrange("p (s b) -> p s b", b=NB)
                    nc.scalar.activation(dst, src, AF.Copy)

            def emit_xg1_chunk(c):
                """xg1 for steps [c*C, (c+1)*C) from h1 history (2 K-chunks)."""
                base_step = (c * C) % S
                seg = h1f[:, ds(base_step * W, C * W)]
                seg = seg.rearrange("p (s k b) -> p k s b", k=2, b=NB)
                for m in range(6):
                    ps = pbig.tile([128, C * NB], F32, tag="big")
                    for k in range(2):
                        nc.tensor.matmul(ps[:],
                                         wih1[:, ds(k * G + m * 128, 128)],
                                         seg[:, k],
                                         start=(k == 0), stop=(k == 1))
                    dst = xg1[:, ds(base_step * GW, C * GW)]
                    dst = dst.rearrange("p (s g) -> p s g", g=GW)
                    dst = dst[:, :, ds(m * NB, NB)]
                    src = ps[:].rearrange("p (s b) -> p s b", b=NB)
                    nc.scalar.activation(dst, src, AF.Copy)

            def emit_regressor_chunk(rc):
                """relu(h2@W1.T+b1) @ W2.T + b2 -> relu -> out, for steps
                [rc*C, (rc+1)*C) of layer 1."""
                base_step = (rc * C) % S
                seg = h2f[:, ds(base_step * W, C * W)]
                seg = seg.rearrange("p (s k b) -> p k s b", k=2, b=NB)
                rT = work.tile([128, 2 * C * NB], F32, tag="rT")
                for mm in range(2):
                    ps = pbig.tile([128, C * NB], F32, tag="big")
                    for k in range(2):
                        nc.tensor.matmul(ps[:],
                                         w1[:, ts(mm * 2 + k, 128)],
                                         seg[:, k],
                                         start=(k == 0), stop=(k == 1))
                    nc.scalar.activation(rT[:, ts(mm, C * NB)], ps[:],
                                         AF.Relu, bias=b1[:, ds(mm, 1)])
                po = pst2.tile([1, C * NB], F32, tag="st2")
                for k in range(2):
                    nc.tensor.matmul(po[:], w2[:, ds(k, 1)],
                                     rT[:, ts(k, C * NB)],
                                     start=(k == 0), stop=(k == 1))
                oT = work.tile([1, C * NB], F32, tag="oT")
                nc.scalar.activation(oT[:], po[:], AF.Relu, bias=b2[:, ds(0, 1)])
                nc.sync.dma_start(out_h[ds(rc, 1), :], oT[:])

            def step_gen(layer, t):
                """Generator emitting one GRU step; yields between chain ops so
                the two layers' chains can be interleaved per-engine."""
                wh = wh0 if layer == 0 else wh1
                hist = h1h if layer == 0 else h2h
                hf = h1f if layer == 0 else h2f
                xg = xg0 if layer == 0 else xg1
                prev = (t - 1) % S
                cur = t % S
                xb = cur * GW
                ps = pg.tile([128, GW], F32, tag=f"g{layer}")
                for m in range(6):
                    for k in range(2):
                        nc.tensor.matmul(ps[:, ts(m, NB)],
                                         wh[:, ts(m * 2 + k, 128)],
                                         hist[:, ds(prev * W + k * NB, NB)],
                                         start=(k == 0), stop=(k == 1))
                yield
                # rz pre-activation += xg ; sigmoid
                nc.vector.tensor_add(ps[:, 0:4 * NB], ps[:, 0:4 * NB],
                                     xg[:, ds(xb, 4 * NB)])
                yield
                rz = work.tile([128, 4 * NB], F32, tag=f"rz{layer}")
                nc.scalar.activation(rz[:], ps[:, 0:4 * NB], AF.Sigmoid)
                yield
                # n = tanh(xn + r*hn)
                tt = work.tile([128, 2 * NB], F32, tag=f"tt{layer}")
                nc.vector.tensor_mul(tt[:], rz[:, 0:2 * NB], ps[:, ds(4 * NB, 2 * NB)])
                yield
                t2 = work.tile([128, 2 * NB], F32, tag=f"t2{layer}")
                nc.vector.tensor_add(t2[:], tt[:], xg[:, ds(xb + 4 * NB, 2 * NB)])
                yield
                nn = work.tile([128, 2 * NB], F32, tag=f"nn{layer}")
                nc.scalar.activation(nn[:], t2[:], AF.Tanh)
                yield
                # h' = n + z*(h - n)
                dd = work.tile([128, 2 * NB], F32, tag=f"dd{layer}")
                nc.vector.tensor_sub(dd[:], hf[:, ds(prev * W, W)], nn[:])
                yield
                zd = work.tile([128, 2 * NB], F32, tag=f"zd{layer}")
                nc.vector.tensor_mul(zd[:], rz[:, ds(2 * NB, 2 * NB)], dd[:])
                yield
                nc.vector.tensor_add(hf[:, ds(cur * W, W)], nn[:], zd[:])
                if state_f32:
                    yield
                    nc.vector.tensor_copy(hist[:, ds(cur * W, W)],
                                          hf[:, ds(cur * W, W)])

            def emit_round(r):
                gens = []
                if r < T:
                    gens.append(step_gen(0, r))
                if r >= D:
                    gens.append(step_gen(1, r - D))
                while gens:
                    gens = [g for g in gens if next(g, "done") != "done"]

            no_aux = os.environ.get("KNOAUX", "0") == "1"

            def emit_body():
                # zero initial h slots (slot S-1 == slot(-1))
                nc.vector.memset(h1h[:, ds((S - 1) * W, W)], 0.0)
                nc.vector.memset(h2h[:, ds((S - 1) * W, W)], 0.0)
                if state_f32:
                    nc.vector.memset(h1f[:, ds((S - 1) * W, W)], 0.0)
                    nc.vector.memset(h2f[:, ds((S - 1) * W, W)], 0.0)
                if no_aux:
                    nc.vector.memset(xg0[:], 0.01)
                    nc.vector.memset(xg1[:], 0.01)
                else:
                    emit_xg0_chunk(0)
                    emit_xg0_chunk(1)
                n_rounds = T + D
                for r in range(n_rounds):
                    emit_round(r)
                    if no_aux:
                        continue
                    if r < T and (r + 1) % C == 0:
                        c = (r + 1) // C - 1  # chunk of layer-0 just finished
                        if c + 2 < T // C:
                            emit_xg0_chunk(c + 2)
                        emit_xg1_chunk(c)
                    if r >= D and (r - D + 1) % C == 0:
                        emit_regressor_chunk((r - D + 1) // C - 1)
                if no_aux:
                    emit_regressor_chunk(0)

            if repeat == 1:
                emit_body()
            else:
                with tc.For_i(0, repeat, 1):
                    emit_body()

    nc.compile()
    return nc


_CACHE = {}


def _get_program(dt=F32, repeat=1):
    key = (str(dt), repeat)
    if key not in _CACHE:
        _CACHE[key] = build_program(dt, repeat)
    return _CACHE[key]


def make_in_maps(inputs, np_dt=np.float32):
    """Host-side prep: slice batch, transpose, pack biases."""
    x = np.asarray(inputs["x"], np.float32)
    Wih0 = np.asarray(inputs["Wih0"], np.float32)
    Whh0 = np.asarray(inputs["Whh0"], np.float32)
    bih0 = np.asarray(inputs["bih0"], np.float32)
    bhh0 = np.asarray(inputs["bhh0"], np.float32)
    Wih1 = np.asarray(inputs["Wih1"], np.float32)
    Whh1 = np.asarray(inputs["Whh1"], np.float32)
    bih1 = np.asarray(inputs["bih1"], np.float32)
    bhh1 = np.asarray(inputs["bhh1"], np.float32)
    W1 = np.asarray(inputs["W1"], np.float32)
    b1 = np.asarray(inputs["b1"], np.float32)
    W2 = np.asarray(inputs["W2"], np.float32)
    b2 = np.asarray(inputs["b2"], np.float32)

    assert not np.any(bhh0[2 * H:]) and not np.any(bhh1[2 * H:]), \
        "nonzero bhh n-gate bias not supported by this build"
    assert not np.any(bih1) and not np.any(bhh1[:2 * H]), \
        "nonzero layer-1 input bias not supported by this build"

    bias0 = np.concatenate([bih0[:2 * H] + bhh0[:2 * H], bih0[2 * H:]])
    wih0T = np.vstack([Wih0.T, bias0[None, :]]).astype(np.float32)  # [17, 768]

    shared = {
        "wh0T": Whh0.T.copy().astype(np_dt),
        "wih0T": wih0T,
        "wh1T": Whh1.T.copy().astype(np_dt),
        "wih1T": Wih1.T.copy().astype(np.float32),
        "w1T": W1.T.copy().astype(np.float32),
        "b1c": b1.reshape(2, 128).T.copy().astype(np.float32),
        "w2c": W2[0].reshape(2, 128).T.copy().astype(np.float32),
        "b2c": b2.reshape(1, 1).astype(np.float32),
    }
    in_maps = []
    for c in range(N_CORES):
        xc = x[c * Bc:(c + 1) * Bc]  # [8, T, 16]
        xTc = xc.transpose(2, 1, 0).reshape(I_DIM, T * Bc)  # [16, T*8]
        xTc = np.vstack([xTc, np.ones((1, T * Bc), np.float32)]).astype(np.float32)
        m = dict(shared)
        m["xT"] = xTc
        in_maps.append(m)
    return in_maps


def assemble_output(results):
    outs = []
    for c in range(N_CORES):
        r = np.asarray(results[c]["out"], np.float32)  # [32, 512]
        r = r.reshape(T // C, C, Bc).transpose(2, 0, 1).reshape(Bc, T)
        outs.append(r)
    return np.concatenate(outs, axis=0)[:, :, None]  # [64, 2048, 1]


if os.environ.get("KBF16", "0") == "1":
    import ml_dtypes
    DT_COMPUTE = BF16
    NP_DT = ml_dtypes.bfloat16
else:
    DT_COMPUTE = F32
    NP_DT = np.float32


def kernel(**inputs):
    nc = _get_program(DT_COMPUTE, 1)
    in_maps = make_in_maps(inputs, NP_DT)
    res = run_bass_kernel_spmd(nc, in_maps, core_ids=list(range(N_CORES)))
    return assemble_output(res.results)


if __name__ == "__main__":
    np.random.seed(0)
    ins = {
        "x": np.random.randn(B_TOTAL, T, I_DIM).astype(np.float32),
    }



# revision 9
# speedup vs baseline: 3.6768x; 3.6768x over previous
"""Trainium2 Bass kernel for a 2-layer GRU (B=64, T=2048, I=16, H=256) + MLP regressor.

v2 strategy:
  - Data parallel: batch 64 sharded as 8 sequences per NeuronCore.
  - Both GRU layers per core, skewed by D=128 steps, with their per-step
    elementwise chains FUSED into one wide op sequence per round:
    one PSUM tile [128, 128] holds [r_L0|r_L1|z_L0|z_L1|xn_L0|xn_L1|hn_L0|hn_L1]
    (16 cols each), so the round chain is:
      identity-MM preloads x-gates -> 24 gate MMs accumulate ->
      sigmoid(64) -> mul(32) -> add(32) -> tanh(32) -> sub -> mul -> add -> cast
    = 1 PE burst + 2 ACT + 6 DVE ops, ~5 cross-engine hops per round
    (vs ~16 ops / ~15 hops in v1).
  - All matmul operands bf16 (FWL halves LDWEIGHTS, which dominates real PE
    time at N=8); fp32 master h state, PSUM accumulation fp32.
  - Input-gate projections precomputed in C-step chunks into a bf16 ring with
    the fused per-step layout; regressor fused every C steps.
"""

import os
import sys

import numpy as np

if "/opt/trn_rl_repo" not in sys.path:
    sys.path.insert(0, "/opt/trn_rl_repo")

import concourse.bacc as bacc
import concourse.mybir as mybir
import concourse.tile as tile
from concourse.bass import ds, ts
from concourse.bass_utils import run_bass_kernel_spmd

# Problem constants (hardcoded per harness contract)
B_TOTAL = 64
N_CORES = 8
Bc = B_TOTAL // N_CORES  # 8 sequences per core
T = 2048
I_DIM = 16
H = 256
G = 3 * H  # 768 gate rows
C = 64  # chunk size for batched precomputes
S = 128  # ring size in steps (2 chunks)
D = 128  # layer-1 skew (steps)

F32 = mybir.dt.float32
BF16 = mybir.dt.bfloat16

AF = mybir.ActivationFunctionType

NB = Bc              # batch per core
W = 4 * NB           # 32: h-state cols per step  [L0k0|L0k1|L1k0|L1k1]
# xg ring cols per step: [xr0|xr1|xz0|xz1|xn0|xn1|zeros]; the zeros block is
# copied by the identity-MM into the hn region so the whole PSUM accumulation
# group is started by one start=True matmul.
GW = 16 * NB         # 128

# per-step psum column offsets
PS_R = 0             # r gates, both layers   (cols  0..31)
PS_Z = 4 * NB        # z gates, both layers   (cols 32..63)
PS_XN = 8 * NB       # xn preload             (cols 64..95)
PS_HN = 12 * NB      # hn (recurrent n part)  (cols 96..127)


def _ps_col(layer, g, m):
    """psum col offset for gate g in {'r','z','n'}, chunk m in {0,1}."""
    base = {"r": PS_R, "z": PS_Z, "n": PS_HN}[g]
    return base + layer * 2 * NB + m * NB


def _ring_col(layer, g, m):
    """xg ring per-step col offset; 'n' here is the xn slot."""
    base = {"r": 0, "z": 4 * NB, "n": 8 * NB}[g]
    return base + layer * 2 * NB + m * NB


def build_program(dt_compute=BF16, repeat=1):
    """Build + compile the SPMD program (identical on all 8 cores)."""
    DT = dt_compute
    nc = bacc.Bacc("TRN2", target_bir_lowering=False, debug=False,
                   num_devices=N_CORES)

    # ---- DRAM I/O ----
    xT_h = nc.dram_tensor("xT", [I_DIM + 1, T * Bc], DT, kind="ExternalInput")
    wh0_h = nc.dram_tensor("wh0T", [H, G], DT, kind="ExternalInput")
    wih0_h = nc.dram_tensor("wih0T", [I_DIM + 1, G], DT, kind="ExternalInput")
    wh1_h = nc.dram_tensor("wh1T", [H, G], DT, kind="ExternalInput")
    wih1_h = nc.dram_tensor("wih1T", [H, G], DT, kind="ExternalInput")
    ident_h = nc.dram_tensor("ident", [128, 128], DT, kind="ExternalInput")
    w1_h = nc.dram_tensor("w1T", [H, H], DT, kind="ExternalInput")
    b1_h = nc.dram_tensor("b1c", [128, 2], F32, kind="ExternalInput")
    w2_h = nc.dram_tensor("w2c", [128, 2], DT, kind="ExternalInput")
    b2_h = nc.dram_tensor("b2c", [1, 1], F32, kind="ExternalInput")
    out_h = nc.dram_tensor("out", [T // C, C * Bc], F32, kind="ExternalOutput")

    with tile.TileContext(nc) as tc:
        with (
            tc.tile_pool(name="cst", bufs=1) as cst,
            tc.tile_pool(name="work", bufs=3) as work,
            tc.tile_pool(name="pg", bufs=2, space="PSUM") as pg,
            tc.tile_pool(name="pbig", bufs=3, space="PSUM") as pbig,
            tc.tile_pool(name="pst2", bufs=1, space="PSUM") as pst2,
        ):
            # ---- persistent SBUF ----
            xT = cst.tile([I_DIM + 1, T * NB], DT, tag="xT")
            wh0 = cst.tile([128, 12 * 128], DT, tag="wh0")
            wh1 = cst.tile([128, 12 * 128], DT, tag="wh1")
            wih0 = cst.tile([I_DIM + 1, G], DT, tag="wih0")
            wih1 = cst.tile([128, 2 * G], DT, tag="wih1")
            ident = cst.tile([128, 128], DT, tag="ident")
            w1 = cst.tile([128, 4 * 128], DT, tag="w1")
            w2 = cst.tile([128, 2], DT, tag="w2")
            b1 = cst.tile([128, 2], F32, tag="b1")
            b2 = cst.tile([1, 1], F32, tag="b2")
            xg = cst.tile([128, S * GW], DT, tag="xg")      # fused xg ring
            hist = cst.tile([128, S * W], DT, tag="hist")   # bf16 h (mm operand)
            hf = cst.tile([128, S * W], F32, tag="hf")      # fp32 master h

            # ---- load constants ----
            nc.sync.dma_start(xT[:], xT_h[:, :])
            for m in range(6):
                for k in range(2):
                    i = m * 2 + k
                    nc.sync.dma_start(wh0[:, ts(i, 128)],
                                      wh0_h[ds(k * 128, 128), ds(m * 128, 128)])
                    nc.sync.dma_start(wh1[:, ts(i, 128)],
                                      wh1_h[ds(k * 128, 128), ds(m * 128, 128)])
            nc.sync.dma_start(wih0[:], wih0_h[:, :])
            for k in range(2):
                nc.sync.dma_start(wih1[:, ts(k, G)], wih1_h[ds(k * 128, 128), :])
            nc.sync.dma_start(ident[:], ident_h[:, :])
            for mm in range(2):
                for k in range(2):
                    nc.sync.dma_start(w1[:, ts(mm * 2 + k, 128)],
                                      w1_h[ds(k * 128, 128), ds(mm * 128, 128)])
            nc.sync.dma_start(w2[:], w2_h[:, :])
            nc.sync.dma_start(b1[:], b1_h[:, :])
            nc.sync.dma_start(b2[:], b2_h[:, :])
            # One-time ring clear: establishes the permanent zeros block
            # (cols 96..127 of each step) and avoids NaN reads from
            # uninitialized layer-1 columns in early rounds.
            nc.vector.memset(xg[:], 0.0)

            def hist_mv(layer, t, k):
                """moving operand: h_{layer}(t) k-chunk, [128, NB] bf16."""
                slot = t % S
                return hist[:, ds(slot * W + (2 * layer + k) * NB, NB)]

            def emit_xg0_chunk(c):
                """layer-0 x-gates for steps [c*C, (c+1)*C)."""
                base_step = (c * C) % S
                for m in range(6):
                    g, j = ("r", "z", "n")[m // 2], m % 2
                    ps = pbig.tile([128, C * NB], F32, tag="big")
                    nc.tensor.matmul(ps[:],
                                     wih0[:, ts(m, 128)],
                                     xT[:, ds(c * C * NB, C * NB)],
                                     start=True, stop=True)
                    dst = xg[:, ds(base_step * GW, C * GW)]
                    dst = dst.rearrange("p (s g) -> p s g", g=GW)
                    dst = dst[:, :, ds(_ring_col(0, g, j), NB)]
                    src = ps[:].rearrange("p (s b) -> p s b", b=NB)
                    nc.scalar.activation(dst, src, AF.Copy)

            def emit_xg1_chunk(c):
                """layer-1 x-gates for steps [c*C, (c+1)*C) from the layer-0
                h history (cols 0..15 of each step block)."""
                base_step = (c * C) % S
                seg = hist[:, ds(base_step * W, C * W)]
                seg = seg.rearrange("p (s c) -> p s c", c=W)
                for m in range(6):
                    g, j = ("r", "z", "n")[m // 2], m % 2
                    ps = pbig.tile([128, C * NB], F32, tag="big")
                    for k in range(2):
                        nc.tensor.matmul(ps[:],
                                         wih1[:, ds(k * G + m * 128, 128)],
                                         seg[:, :, ds(k * NB, NB)],
                                         start=(k == 0), stop=(k == 1))
                    dst = xg[:, ds(base_step * GW, C * GW)]
                    dst = dst.rearrange("p (s g) -> p s g", g=GW)
                    dst = dst[:, :, ds(_ring_col(1, g, j), NB)]
                    src = ps[:].rearrange("p (s b) -> p s b", b=NB)
                    nc.scalar.activation(dst, src, AF.Copy)

            def emit_regressor_chunk(rc):
                """relu(h2@W1.T+b1) @ W2.T + b2 -> relu -> out for steps
                [rc*C, (rc+1)*C) of layer 1."""
                base_step = (rc * C) % S
                seg = hist[:, ds(base_step * W, C * W)]
                seg = seg.rearrange("p (s c) -> p s c", c=W)
                rT = work.tile([128, 2 * C * NB], DT, tag="rT")
                for mm in range(2):
                    ps = pbig.tile([128, C * NB], F32, tag="big")
                    for k in range(2):
                        nc.tensor.matmul(ps[:],
                                         w1[:, ts(mm * 2 + k, 128)],
                                         seg[:, :, ds((2 + k) * NB, NB)],
                                         start=(k == 0), stop=(k == 1))
                    nc.scalar.activation(rT[:, ts(mm, C * NB)], ps[:],
                                         AF.Relu, bias=b1[:, ds(mm, 1)])
                po = pst2.tile([1, C * NB], F32, tag="st2")
                for k in range(2):
                    nc.tensor.matmul(po[:], w2[:, ds(k, 1)],
                                     rT[:, ts(k, C * NB)],
                                     start=(k == 0), stop=(k == 1))
                oT = work.tile([1, C * NB], F32, tag="oT")
                nc.scalar.activation(oT[:], po[:], AF.Relu, bias=b2[:, ds(0, 1)])
                nc.sync.dma_start(out_h[ds(rc, 1), :], oT[:])

            def emit_round(r):
                """One fused round: layer0 step r (if r < T) and layer1 step
                r - D (if r >= D). Since D == S, both active layers always
                share the same ring slot index."""
                layers = []
                if r < T:
                    layers.append((0, r))
                if r >= D:
                    layers.append((1, r - D))
                both = len(layers) == 2
                slot = layers[0][1] % S
                assert all(t % S == slot for _, t in layers)

                ps = pg.tile([128, 128], F32, tag="ps")

                # 1) identity-MM preloads x-gates (xr|xz|xn|zeros) into PSUM,
                # starting the whole accumulation group (incl. the hn region,
                # which gets the ring's permanent zeros block). In single-layer
                # rounds the inactive layer's columns receive stale ring data;
                # they are never read.
                nc.tensor.matmul(ps[:, 0:GW],
                                 ident[:, :],
                                 xg[:, ds(slot * GW, GW)],
                                 start=True, stop=False)

                # 2) gate MMs: rz for all active layers first, then n.
                # All share ONE psum accumulation bracket (started by the
                # identity-MM); only the final MM carries stop=True.
                mms = []
                for g in ("r", "z"):
                    for layer, t in layers:
                        moff = 0 if g == "r" else 2
                        for m in range(2):
                            for k in range(2):
                                mms.append((layer, g, m, moff + m, k, t))
                for layer, t in layers:
                    for m in range(2):
                        for k in range(2):
                            mms.append((layer, "n", m, 4 + m, k, t))
                for i, (layer, g, m, mrow, k, t) in enumerate(mms):
                    wh = wh0 if layer == 0 else wh1
                    nc.tensor.matmul(
                        ps[:, ds(_ps_col(layer, g, m), NB)],
                        wh[:, ts(mrow * 2 + k, 128)],
                        hist_mv(layer, t - 1, k),
                        start=False, stop=(i == len(mms) - 1))

                # 3) fused elementwise chain.
                # Column views: with both layers the r/z/xn/hn blocks are each
                # contiguous (2 layers x 16); single-layer rounds use the
                # layer's 16-wide sub-slices.
                if both:
                    lo = 0
                    nw = 4 * NB           # 32: n/h width (both layers)
                else:
                    lo = layers[0][0] * 2 * NB
                    nw = 2 * NB

                rz = work.tile([128, 8 * NB], F32, tag="rz")
                if both:
                    nc.scalar.activation(rz[:, 0:8 * NB], ps[:, 0:8 * NB],
                                         AF.Sigmoid)
                else:
                    nc.scalar.activation(rz[:, ds(PS_R + lo, nw)],
                                         ps[:, ds(PS_R + lo, nw)], AF.Sigmoid)
                    nc.scalar.activation(rz[:, ds(PS_Z + lo, nw)],
                                         ps[:, ds(PS_Z + lo, nw)], AF.Sigmoid)
                tt = work.tile([128, 4 * NB], F32, tag="tt")
                nc.vector.tensor_mul(tt[:, 0:nw], rz[:, ds(PS_R + lo, nw)],
                                     ps[:, ds(PS_HN + lo, nw)])
                t2 = work.tile([128, 4 * NB], F32, tag="t2")
                nc.vector.tensor_add(t2[:, 0:nw], tt[:, 0:nw],
                                     ps[:, ds(PS_XN + lo, nw)])
                nn = work.tile([128, 4 * NB], F32, tag="nn")
                nc.scalar.activation(nn[:, 0:nw], t2[:, 0:nw], AF.Tanh)

                # h' = n + z*(h_prev - n)
                prev, cur = (layers[0][1] - 1) % S, slot
                hprev = hf[:, ds(prev * W + lo, nw)]
                hcur = hf[:, ds(cur * W + lo, nw)]
                hcur_b = hist[:, ds(cur * W + lo, nw)]
                zsl = rz[:, ds(PS_Z + lo, nw)]
                dd = work.tile([128, 4 * NB], F32, tag="dd")
                nc.vector.tensor_sub(dd[:, 0:nw], hprev, nn[:, 0:nw])
                zd = work.tile([128, 4 * NB], F32, tag="zd")
                nc.vector.tensor_mul(zd[:, 0:nw], zsl, dd[:, 0:nw])
                nc.vector.tensor_add(hcur, nn[:, 0:nw], zd[:, 0:nw])
                nc.vector.tensor_copy(hcur_b, hcur)

            no_aux = os.environ.get("KNOAUX", "0") == "1"

            def emit_body():
                # zero initial h slots (slot S-1 == slot(-1))
                nc.vector.memset(hf[:, ds((S - 1) * W, W)], 0.0)
                nc.vector.memset(hist[:, ds((S - 1) * W, W)], 0.0)
                if no_aux:
                    nc.vector.memset(xg[:], 0.01)
                else:
                    emit_xg0_chunk(0)
                    emit_xg0_chunk(1)
                n_rounds = T + D
                for r in range(n_rounds):
                    emit_round(r)
                    if no_aux:
                        continue
                    if r < T and (r + 1) % C == 0:
                        c = (r + 1) // C - 1  # layer-0 chunk just finished
                        if c + 2 < T // C:
                            emit_xg0_chunk(c + 2)
                        emit_xg1_chunk(c)
                    if r >= D and (r - D + 1) % C == 0:
                        emit_regressor_chunk((r - D + 1) // C - 1)
                if no_aux:
                    emit_regressor_chunk(0)

            if repeat == 1:
                emit_body()
            else:
                with tc.For_i(0, repeat, 1):
                    emit_body()

    nc.compile()
    return nc


_CACHE = {}


def _get_program(dt=BF16, repeat=1):
    key = (str(dt), repeat)
    if key not in _CACHE:
        _CACHE[key] = build_program(dt, repeat)
    return _CACHE[key]


def make_in_maps(inputs, np_dt=None):
    """Host-side prep: slice batch, transpose, pack biases, cast bf16."""
    import ml_dtypes
    if np_dt is None:
        np_dt = ml_dtypes.bfloat16
    x = np.asarray(inputs["x"], np.float32)
    Wih0 = np.asarray(inputs["Wih0"], np.float32)
    Whh0 = np.asarray(inputs["Whh0"], np.float32)
    bih0 = np.asarray(inputs["bih0"], np.float32)
    bhh0 = np.asarray(inputs["bhh0"], np.float32)
    Wih1 = np.asarray(inputs["Wih1"], np.float32)
    Whh1 = np.asarray(inputs["Whh1"], np.float32)
    bih1 = np.asarray(inputs["bih1"], np.float32)
    bhh1 = np.asarray(inputs["bhh1"], np.float32)
    W1 = np.asarray(inputs["W1"], np.float32)
    b1 = np.asarray(inputs["b1"], np.float32)
    W2 = np.asarray(inputs["W2"], np.float32)
    b2 = np.asarray(inputs["b2"], np.float32)

    assert not np.any(bhh0[2 * H:]) and not np.any(bhh1[2 * H:]), \
        "nonzero bhh n-gate bias not supported by this build"
    assert not np.any(bih1) and not np.any(bhh1[:2 * H]), \
        "nonzero layer-1 input bias not supported by this build"

    bias0 = np.concatenate([bih0[:2 * H] + bhh0[:2 * H], bih0[2 * H:]])
    wih0T = np.vstack([Wih0.T, bias0[None, :]]).astype(np_dt)  # [17, 768]

    shared = {
        "wh0T": Whh0.T.copy().astype(np_dt),
        "wih0T": wih0T,
        "wh1T": Whh1.T.copy().astype(np_dt),
        "wih1T": Wih1.T.copy().astype(np_dt),
        "ident": np.eye(128, dtype=np_dt),
        "w1T": W1.T.copy().astype(np_dt),
        "b1c": b1.reshape(2, 128).T.copy().astype(np.float32),
        "w2c": W2[0].reshape(2, 128).T.copy().astype(np_dt),
        "b2c": b2.reshape(1, 1).astype(np.float32),
    }
    in_maps = []
    for c in range(N_CORES):
        xc = x[c * Bc:(c + 1) * Bc]  # [8, T, 16]
        xTc = xc.transpose(2, 1, 0).reshape(I_DIM, T * Bc)  # [16, T*8]
        xTc = np.vstack([xTc, np.ones((1, T * Bc), np.float32)]).astype(np_dt)
        m = dict(shared)
        m["xT"] = xTc
        in_maps.append(m)
    return in_maps


def assemble_output(results):
    outs = []
    for c in range(N_CORES):
        r = np.asarray(results[c]["out"], np.float32)  # [32, 512]
        r = r.reshape(T // C, C, Bc).transpose(2, 0, 1).reshape(Bc, T)
        outs.append(r)
    return np.concatenate(outs, axis=0)[:, :, None]  # [64, 2048, 1]


DT_COMPUTE = BF16
NP_DT = None  # resolved to ml_dtypes.bfloat16 in make_in_maps


def kernel(**inputs):
    nc = _get_program(DT_COMPUTE, 1)
    in_maps = make_in_maps(inputs)
    res = run_bass_kernel_spmd(nc, in_maps, core_ids=list(range(N_CORES)))
    return assemble_output(res.results)


# revision 10
# speedup vs baseline: 5.9015x; 1.6050x over previous
"""Trainium2 Bass kernel for a 2-layer GRU (B=64, T=2048, I=16, H=256) + MLP regressor.

v3 strategy:
  - Data parallel: batch 64 sharded as 8 sequences per NeuronCore.
  - Both GRU layers per core, skewed by D=128 steps, each with its OWN
    per-step PSUM tile and a SHORT elementwise chain; the two layers' chains
    are emitted interleaved so layer A's matmul group and ACT ops overlap
    layer B's DVE ops (antiphase 2-stage pipeline on shared engines).
  - Per layer-step: identity-MM preloads [xr|xz|xn|0] into PSUM (starts the
    accumulation bracket; kills the xg add), 12 gate MMs accumulate, then:
      sigmoid(32->bf16) ; [GPSIMD: u=1-z, zh=z*h_prev] ;
      mul(r*hn) ; add(+xn) ; tanh(->bf16) ; mul(n*u) ; add(+zh -> h' bf16)
    The h state lives ONLY in bf16 (it is the matmul moving operand), so
    there is no cast on the serial path.
  - All matmul operands bf16 (FWL halves LDWEIGHTS, which dominates PE time
    at N=8); PSUM accumulation fp32.
  - Input-gate projections precomputed in C-step chunks into a bf16 ring;
    regressor fused every C steps.
"""

import os
import sys

import numpy as np

if "/opt/trn_rl_repo" not in sys.path:
    sys.path.insert(0, "/opt/trn_rl_repo")

import concourse.bacc as bacc
import concourse.mybir as mybir
import concourse.tile as tile
from concourse.bass import ds, ts
from concourse.bass_utils import run_bass_kernel_spmd

# Problem constants (hardcoded per harness contract)
B_TOTAL = 64
N_CORES = 8
Bc = B_TOTAL // N_CORES  # 8 sequences per core
T = 2048
I_DIM = 16
H = 256
G = 3 * H  # 768 gate rows
C = 64  # chunk size for batched precomputes
S = 128  # ring size in steps (2 chunks)
D = 128  # layer-1 skew (steps); D == S so both layers share slot indices

F32 = mybir.dt.float32
BF16 = mybir.dt.bfloat16
AF = mybir.ActivationFunctionType
ALU = mybir.AluOpType

NB = Bc              # 8: batch per core
W = 4 * NB           # 32: h-state cols per step  [L0k0|L0k1|L1k0|L1k1]
LG = 8 * NB          # 64: per-layer ring cols per step [xr|xz|xn|zeros]
GW = 2 * LG          # 128: ring cols per step [L0 | L1]

# per-layer psum layout (64 cols): [r | z | xn | hn]
PS_R, PS_Z, PS_XN, PS_HN = 0, 2 * NB, 4 * NB, 6 * NB


def _ring_col(layer, g, m):
    """xg ring per-step col offset for gate g in {'r','z','n'}, chunk m."""
    return layer * LG + {"r": 0, "z": 2 * NB, "n": 4 * NB}[g] + m * NB


def build_program(dt_compute=BF16, repeat=1):
    """Build + compile the SPMD program (identical on all 8 cores)."""
    DT = dt_compute
    nc = bacc.Bacc("TRN2", target_bir_lowering=False, debug=False,
                   num_devices=N_CORES)

    # ---- DRAM I/O ----
    xT_h = nc.dram_tensor("xT", [I_DIM + 1, T * Bc], DT, kind="ExternalInput")
    wh0_h = nc.dram_tensor("wh0T", [H, G], DT, kind="ExternalInput")
    wih0_h = nc.dram_tensor("wih0T", [I_DIM + 1, G], DT, kind="ExternalInput")
    wh1_h = nc.dram_tensor("wh1T", [H, G], DT, kind="ExternalInput")
    wih1_h = nc.dram_tensor("wih1T", [H, G], DT, kind="ExternalInput")
    ident_h = nc.dram_tensor("ident", [128, 128], DT, kind="ExternalInput")
    w1_h = nc.dram_tensor("w1T", [H, H], DT, kind="ExternalInput")
    b1_h = nc.dram_tensor("b1c", [128, 2], F32, kind="ExternalInput")
    w2_h = nc.dram_tensor("w2c", [128, 2], DT, kind="ExternalInput")
    b2_h = nc.dram_tensor("b2c", [1, 1], F32, kind="ExternalInput")
    out_h = nc.dram_tensor("out", [T // C, C * Bc], F32, kind="ExternalOutput")

    with tile.TileContext(nc) as tc:
        with (
            tc.tile_pool(name="cst", bufs=1) as cst,
            tc.tile_pool(name="work", bufs=3) as work,
            tc.tile_pool(name="pg", bufs=4, space="PSUM") as pg,
            tc.tile_pool(name="pbig", bufs=3, space="PSUM") as pbig,
            tc.tile_pool(name="pst2", bufs=1, space="PSUM") as pst2,
        ):
            # ---- persistent SBUF ----
            xT = cst.tile([I_DIM + 1, T * NB], DT, tag="xT")
            wh0 = cst.tile([128, 12 * 128], DT, tag="wh0")
            wh1 = cst.tile([128, 12 * 128], DT, tag="wh1")
            wih0 = cst.tile([I_DIM + 1, G], DT, tag="wih0")
            wih1 = cst.tile([128, 2 * G], DT, tag="wih1")
            ident = cst.tile([128, 128], DT, tag="ident")
            w1 = cst.tile([128, 4 * 128], DT, tag="w1")
            w2 = cst.tile([128, 2], DT, tag="w2")
            b1 = cst.tile([128, 2], F32, tag="b1")
            b2 = cst.tile([1, 1], F32, tag="b2")
            xg = cst.tile([128, S * GW], DT, tag="xg")      # x-gates ring
            hist = cst.tile([128, S * W], DT, tag="hist")   # bf16 h state

            # ---- load constants ----
            nc.sync.dma_start(xT[:], xT_h[:, :])
            for m in range(6):
                for k in range(2):
                    i = m * 2 + k
                    nc.sync.dma_start(wh0[:, ts(i, 128)],
                                      wh0_h[ds(k * 128, 128), ds(m * 128, 128)])
                    nc.sync.dma_start(wh1[:, ts(i, 128)],
                                      wh1_h[ds(k * 128, 128), ds(m * 128, 128)])
            nc.sync.dma_start(wih0[:], wih0_h[:, :])
            for k in range(2):
                nc.sync.dma_start(wih1[:, ts(k, G)], wih1_h[ds(k * 128, 128), :])
            nc.sync.dma_start(ident[:], ident_h[:, :])
            for mm in range(2):
                for k in range(2):
                    nc.sync.dma_start(w1[:, ts(mm * 2 + k, 128)],
                                      w1_h[ds(k * 128, 128), ds(mm * 128, 128)])
            nc.sync.dma_start(w2[:], w2_h[:, :])
            nc.sync.dma_start(b1[:], b1_h[:, :])
            nc.sync.dma_start(b2[:], b2_h[:, :])
            # One-time ring clear: establishes the permanent zeros blocks and
            # avoids NaN reads from uninitialized columns in early rounds.
            nc.vector.memset(xg[:], 0.0)

            def hist_mv(layer, t, k):
                """moving operand: h_{layer}(t) k-chunk, [128, NB] bf16."""
                slot = t % S
                return hist[:, ds(slot * W + (2 * layer + k) * NB, NB)]

            def hist_hcols(layer, t):
                """h_{layer}(t): both k-chunks, [128, 2*NB] bf16."""
                slot = t % S
                return hist[:, ds(slot * W + 2 * layer * NB, 2 * NB)]

            def emit_xg0_chunk(c):
                """layer-0 x-gates for steps [c*C, (c+1)*C)."""
                base_step = (c * C) % S
                for m in range(6):
                    g, j = ("r", "z", "n")[m // 2], m % 2
                    ps = pbig.tile([128, C * NB], F32, tag="big")
                    nc.tensor.matmul(ps[:],
                                     wih0[:, ts(m, 128)],
                                     xT[:, ds(c * C * NB, C * NB)],
                                     start=True, stop=True)
                    dst = xg[:, ds(base_step * GW, C * GW)]
                    dst = dst.rearrange("p (s g) -> p s g", g=GW)
                    dst = dst[:, :, ds(_ring_col(0, g, j), NB)]
                    src = ps[:].rearrange("p (s b) -> p s b", b=NB)
                    nc.scalar.activation(dst, src, AF.Copy)

            def emit_xg1_chunk(c):
                """layer-1 x-gates for steps [c*C, (c+1)*C) from the layer-0
                h history (cols 0..15 of each step block)."""
                base_step = (c * C) % S
                seg = hist[:, ds(base_step * W, C * W)]
                seg = seg.rearrange("p (s c) -> p s c", c=W)
                for m in range(6):
                    g, j = ("r", "z", "n")[m // 2], m % 2
                    ps = pbig.tile([128, C * NB], F32, tag="big")
                    for k in range(2):
                        nc.tensor.matmul(ps[:],
                                         wih1[:, ds(k * G + m * 128, 128)],
                                         seg[:, :, ds(k * NB, NB)],
                                         start=(k == 0), stop=(k == 1))
                    dst = xg[:, ds(base_step * GW, C * GW)]
                    dst = dst.rearrange("p (s g) -> p s g", g=GW)
                    dst = dst[:, :, ds(_ring_col(1, g, j), NB)]
                    src = ps[:].rearrange("p (s b) -> p s b", b=NB)
                    nc.scalar.activation(dst, src, AF.Copy)

            def emit_regressor_chunk(rc):
                """relu(h2@W1.T+b1) @ W2.T + b2 -> relu -> out for steps
                [rc*C, (rc+1)*C) of layer 1."""
                base_step = (rc * C) % S
                seg = hist[:, ds(base_step * W, C * W)]
                seg = seg.rearrange("p (s c) -> p s c", c=W)
                rT = work.tile([128, 2 * C * NB], DT, tag="rT")
                for mm in range(2):
                    ps = pbig.tile([128, C * NB], F32, tag="big")
                    for k in range(2):
                        nc.tensor.matmul(ps[:],
                                         w1[:, ts(mm * 2 + k, 128)],
                                         seg[:, :, ds((2 + k) * NB, NB)],
                                         start=(k == 0), stop=(k == 1))
                    nc.scalar.activation(rT[:, ts(mm, C * NB)], ps[:],
                                         AF.Relu, bias=b1[:, ds(mm, 1)])
                po = pst2.tile([1, C * NB], F32, tag="st2")
                for k in range(2):
                    nc.tensor.matmul(po[:], w2[:, ds(k, 1)],
                                     rT[:, ts(k, C * NB)],
                                     start=(k == 0), stop=(k == 1))
                oT = work.tile([1, C * NB], F32, tag="oT")
                nc.scalar.activation(oT[:], po[:], AF.Relu, bias=b2[:, ds(0, 1)])
                nc.sync.dma_start(out_h[ds(rc, 1), :], oT[:])

            def emit_mm_group(layer, t):
                """identity preload + 12 gate MMs for one layer-step; returns
                the psum tile."""
                slot = t % S
                wh = wh0 if layer == 0 else wh1
                ps = pg.tile([128, 8 * NB], F32, tag="ps")
                nc.tensor.matmul(ps[:],
                                 ident[:, :],
                                 xg[:, ds(slot * GW + layer * LG, LG)],
                                 start=True, stop=False)
                mms = [(g, m, k)
                       for g in ("r", "z", "n") for m in range(2)
                       for k in range(2)]
                for i, (g, m, k) in enumerate(mms):
                    mrow = {"r": 0, "z": 2, "n": 4}[g] + m
                    pcol = {"r": PS_R, "z": PS_Z, "n": PS_HN}[g] + m * NB
                    nc.tensor.matmul(
                        ps[:, ds(pcol, NB)],
                        wh[:, ts(mrow * 2 + k, 128)],
                        hist_mv(layer, t - 1, k),
                        start=False, stop=(i == len(mms) - 1))
                return ps

            def chain_gen(layer, t, ps):
                """Short per-layer elementwise chain; yields between ops so
                two layers' chains interleave in emission order."""
                rz = work.tile([128, 4 * NB], BF16, tag=f"rz{layer}")
                nc.scalar.activation(rz[:], ps[:, 0:4 * NB], AF.Sigmoid)
                yield
                # z-path on GPSIMD (off the serial path, runs during tanh)
                u = work.tile([128, 2 * NB], BF16, tag=f"u{layer}")
                nc.gpsimd.tensor_scalar(u[:], rz[:, ds(2 * NB, 2 * NB)],
                                        -1.0, 1.0, ALU.mult, ALU.add)
                zh = work.tile([128, 2 * NB], F32, tag=f"zh{layer}")
                nc.gpsimd.tensor_mul(zh[:], rz[:, ds(2 * NB, 2 * NB)],
                                     hist_hcols(layer, t - 1))
                yield
                tt = work.tile([128, 2 * NB], F32, tag=f"tt{layer}")
                nc.vector.tensor_mul(tt[:], rz[:, ds(0, 2 * NB)],
                                     ps[:, ds(PS_HN, 2 * NB)])
                yield
                t2 = work.tile([128, 2 * NB], F32, tag=f"t2{layer}")
                nc.vector.tensor_add(t2[:], tt[:], ps[:, ds(PS_XN, 2 * NB)])
                yield
                nn = work.tile([128, 2 * NB], BF16, tag=f"nn{layer}")
                nc.scalar.activation(nn[:], t2[:], AF.Tanh)
                yield
                nu = work.tile([128, 2 * NB], F32, tag=f"nu{layer}")
                nc.vector.tensor_mul(nu[:], nn[:], u[:])
                yield
                nc.vector.tensor_add(hist_hcols(layer, t), nu[:], zh[:])

            def emit_round(r):
                work_items = []
                if r < T:
                    work_items.append((0, r))
                if r >= D:
                    work_items.append((1, r - D))
                gens = []
                for layer, t in work_items:
                    ps = emit_mm_group(layer, t)
                    gens.append(chain_gen(layer, t, ps))
                while gens:
                    gens = [g for g in gens if next(g, "done") != "done"]

            no_aux = os.environ.get("KNOAUX", "0") == "1"

            def emit_body():
                # zero initial h slots (slot S-1 == slot(-1))
                nc.vector.memset(hist[:, ds((S - 1) * W, W)], 0.0)
                if no_aux:
                    nc.vector.memset(xg[:], 0.01)
                else:
                    emit_xg0_chunk(0)
                    emit_xg0_chunk(1)
                n_rounds = T + D
                for r in range(n_rounds):
                    emit_round(r)
                    if no_aux:
                        continue
                    if r < T and (r + 1) % C == 0:
                        c = (r + 1) // C - 1  # layer-0 chunk just finished
                        if c + 2 < T // C:
                            emit_xg0_chunk(c + 2)
                        emit_xg1_chunk(c)
                    if r >= D and (r - D + 1) % C == 0:
                        emit_regressor_chunk((r - D + 1) // C - 1)
                if no_aux:
                    emit_regressor_chunk(0)

            if repeat == 1:
                emit_body()
            else:
                with tc.For_i(0, repeat, 1):
                    emit_body()

    nc.compile()
    return nc


_CACHE = {}


def _get_program(dt=BF16, repeat=1):
    key = (str(dt), repeat)
    if key not in _CACHE:
        _CACHE[key] = build_program(dt, repeat)
    return _CACHE[key]


def make_in_maps(inputs, np_dt=None):
    """Host-side prep: slice batch, transpose, pack biases, cast bf16."""
    import ml_dtypes
    if np_dt is None:
        np_dt = ml_dtypes.bfloat16
    x = np.asarray(inputs["x"], np.float32)
    Wih0 = np.asarray(inputs["Wih0"], np.float32)
    Whh0 = np.asarray(inputs["Whh0"], np.float32)
    bih0 = np.asarray(inputs["bih0"], np.float32)
    bhh0 = np.asarray(inputs["bhh0"], np.float32)
    Wih1 = np.asarray(inputs["Wih1"], np.float32)
    Whh1 = np.asarray(inputs["Whh1"], np.float32)
    bih1 = np.asarray(inputs["bih1"], np.float32)
    bhh1 = np.asarray(inputs["bhh1"], np.float32)
    W1 = np.asarray(inputs["W1"], np.float32)
    b1 = np.asarray(inputs["b1"], np.float32)
    W2 = np.asarray(inputs["W2"], np.float32)
    b2 = np.asarray(inputs["b2"], np.float32)

    assert not np.any(bhh0[2 * H:]) and not np.any(bhh1[2 * H:]), \
        "nonzero bhh n-gate bias not supported by this build"
    assert not np.any(bih1) and not np.any(bhh1[:2 * H]), \
        "nonzero layer-1 input bias not supported by this build"

    bias0 = np.concatenate([bih0[:2 * H] + bhh0[:2 * H], bih0[2 * H:]])
    wih0T = np.vstack([Wih0.T, bias0[None, :]]).astype(np_dt)  # [17, 768]

    shared = {
        "wh0T": Whh0.T.copy().astype(np_dt),
        "wih0T": wih0T,
        "wh1T": Whh1.T.copy().astype(np_dt),
        "wih1T": Wih1.T.copy().astype(np_dt),
        "ident": np.eye(128, dtype=np_dt),
        "w1T": W1.T.copy().astype(np_dt),
        "b1c": b1.reshape(2, 128).T.copy().astype(np.float32),
        "w2c": W2[0].reshape(2, 128).T.copy().astype(np_dt),
        "b2c": b2.reshape(1, 1).astype(np.float32),
    }
    in_maps = []
    for c in range(N_CORES):
        xc = x[c * Bc:(c + 1) * Bc]  # [8, T, 16]
        xTc = xc.transpose(2, 1, 0).reshape(I_DIM, T * Bc)  # [16, T*8]
        xTc = np.vstack([xTc, np.ones((1, T * Bc), np.float32)]).astype(np_dt)
        m = dict(shared)
        m["xT"] = xTc
        in_maps.append(m)
    return in_maps


def assemble_output(results):
    outs = []
    for c in range(N_CORES):
        r = np.asarray(results[c]["out"], np.float32)  # [32, 512]
        r = r.reshape(T // C, C, Bc).transpose(2, 0, 1).reshape(Bc, T)
        outs.append(r)
    return np.concatenate(outs, axis=0)[:, :, None]  # [64, 2048, 1]


DT_COMPUTE = BF16
NP_DT = None  # resolved to ml_dtypes.bfloat16 in make_in_maps


def kernel(**inputs):
    nc = _get_program(DT_COMPUTE, 1)
    in_maps = make_in_maps(inputs)
    res = run_bass_kernel_spmd(nc, in_maps, core_ids=list(range(N_CORES)))
    return assemble_output(res.results)
